# revision 6
# baseline (speedup 1.0000x reference)
"""DeepKnowledgeTracing (LSTM + per-step MoE expert routing) Trainium2 kernel.

Strategy (8 NeuronCores, tensor-parallel over the 4H gate dimension):
  - Host folds the two tiny input Linears (2->10, 1->10) into the per-expert
    encoder table algebraically:  x[n] = [x1_0, x1_1, x2_0, 1] @ T4[skill[n]]
    with T4[s] = [[m1_W^T @ We_u[s]], [m2_W^T @ We_v[s]], [bias row]]  (exact).
  - Phase A1 (device): per-core indirect-DMA gather of T4 rows for its 1/8
    token shard + block-diagonal batched matmul -> X^T slice [H, 800].
  - Phase A2: AllGather X^T across the 8 cores.
  - Phase A3: gates_x = X @ Wih_k^T for this core's 1024 gate rows, all 6400
    tokens (the state-independent half of the LSTM FLOPs, fully parallel).
  - Phase B: 200-step recurrence. Core k owns 256 h-units (gate rows ordered
    [i|f|o|g]); per step: 16x2 matmuls vs resident Whh_k^T, fused elementwise
    on ACT/DVE, PE-transpose of h_new, 16KB AllGather of h, decoder matmul
    for the step overlapped with the collective.
  - Outputs: full logits [B*T, S] (every core computes them; core 0's used),
    per-core fp32 h/c finals assembled host-side.
"""

import os
import numpy as np
import ml_dtypes

import concourse.bass as bass
import concourse.bacc as bacc
import concourse.mybir as mybir
import concourse.tile as tile
from concourse.bass import IndirectOffsetOnAxis
from concourse.bass_utils import run_bass_kernel_spmd
from concourse.masks import make_identity

N_CORES = 8
B = 32
H = 2048
S = 256
GC = 4 * H // N_CORES   # 1024 gate columns per core (order [i|f|o|g] x 256 units)
U = H // N_CORES        # 256 hidden units per core

_DT_NAME = os.environ.get("DKT_DT", "float32r")
USE_F32R = _DT_NAME == "float32r"
DT = getattr(mybir.dt, _DT_NAME)
NPDT = ml_dtypes.bfloat16 if _DT_NAME == "bfloat16" else np.float32

_BUILD_CACHE = {}


def _build(T):
    TPC = T // N_CORES          # steps handled per core in phase A1
    TOK = TPC * B               # tokens per core in phase A1
    NTOK = T * B
    MT = NTOK // 128            # number of 128-token M-tiles in phase A3
    f32 = mybir.dt.float32

    nc = bacc.Bacc("TRN2", target_bir_lowering=False, debug=False,
                   num_devices=N_CORES)

    # ---- kernel I/O ----
    expt4 = nc.dram_tensor("expt4", [S * 4, H], DT, kind="ExternalInput")
    bd_in = nc.dram_tensor("bd", [128, TOK], DT, kind="ExternalInput")
    offs_in = nc.dram_tensor("offs", [128, TPC], mybir.dt.int32, kind="ExternalInput")
    wihT_in = nc.dram_tensor("wihT", [H, GC], DT, kind="ExternalInput")
    whhT_in = nc.dram_tensor("whhT", [H, GC], DT, kind="ExternalInput")
    bias_in = nc.dram_tensor("biask", [1, GC], f32, kind="ExternalInput")
    decWT_in = nc.dram_tensor("decWT", [H, S], DT, kind="ExternalInput")
    decb_in = nc.dram_tensor("decb", [1, S], f32, kind="ExternalInput")
    h0T_in = nc.dram_tensor("h0T", [H, B], DT, kind="ExternalInput")
    c0_in = nc.dram_tensor("c0k", [B, U], f32, kind="ExternalInput")

    out_d = nc.dram_tensor("out", [NTOK, S], f32, kind="ExternalOutput")
    hout_d = nc.dram_tensor("h_out", [B, U], f32, kind="ExternalOutput")
    cout_d = nc.dram_tensor("c_out", [B, U], f32, kind="ExternalOutput")

    with tile.TileContext(nc) as tc:
        with tc.tile_pool(name="dram", bufs=1, space="DRAM") as dramp, \
             tc.tile_pool(name="dramcc", bufs=4, space="DRAM") as dramcc:
            XT_d = dramp.tile([H, TOK], DT)
            XTfull_d = dramp.tile([N_CORES * H, TOK], DT)
            gx_d = dramp.tile([NTOK, GC], f32)

            # ---------------- Phase A1: expert encode -> X^T ----------------
            with tc.tile_pool(name="a1sb", bufs=1) as a1c, \
                 tc.tile_pool(name="a1g", bufs=3) as a1g, \
                 tc.tile_pool(name="a1ps", bufs=2, space="PSUM") as a1ps:
                bd_sb = a1c.tile([128, TOK], DT)
                nc.sync.dma_start(bd_sb[:], bd_in[:])
                offs_sb = a1c.tile([128, TPC], mybir.dt.int32)
                nc.sync.dma_start(offs_sb[:], offs_in[:])
                xt_stage = a1c.tile([128, 16 * TOK], DT)
                xt_v = xt_stage[:].rearrange("p (pt n) -> p pt n", pt=16)

                for g in range(TPC):
                    gath = a1g.tile([128, H], DT)
                    nc.gpsimd.indirect_dma_start(
                        out=gath[:], out_offset=None,
                        in_=expt4[:],
                        in_offset=IndirectOffsetOnAxis(ap=offs_sb[:, g:g + 1], axis=0),
                    )
                    ps = a1ps.tile([128, 512], f32)
                    for pt in range(16):
                        nc.tensor.matmul(
                            ps[:, 32 * pt:32 * pt + 32],
                            lhsT=(gath[:, 128 * pt:128 * pt + 128]),
                            rhs=(bd_sb[:, 32 * g:32 * g + 32]),
                            start=True, stop=True,
                        )
                    nc.vector.tensor_copy(
                        xt_v[:, :, 32 * g:32 * g + 32],
                        ps[:].rearrange("p (pt j) -> p pt j", pt=16),
                    )
                nc.sync.dma_start(
                    XT_d[:].rearrange("(pt p) n -> p pt n", p=128), xt_stage[:].rearrange("p (pt n) -> p pt n", pt=16)
                )

            # ---------------- Phase A2: AllGather X^T ----------------
            nc.gpsimd.collective_compute(
                "AllGather", mybir.AluOpType.bypass,
                ins=[XT_d.opt()], outs=[XTfull_d.opt()],
                replica_groups=[list(range(N_CORES))],
            )
            XTf_v = XTfull_d[:].rearrange("(rk pt p) n -> p rk pt n", p=128, rk=N_CORES)

            # ---------------- Phase A3: gates_x = X @ WihT_k ----------------
            with tc.tile_pool(name="a3c", bufs=1) as a3c, \
                 tc.tile_pool(name="a3x", bufs=3) as a3x, \
                 tc.tile_pool(name="a3o", bufs=3) as a3o, \
                 tc.tile_pool(name="a3ps", bufs=2, space="PSUM") as a3ps:
                wih_sb = a3c.tile([128, 16 * GC], DT)
                nc.sync.dma_start(
                    wih_sb[:].rearrange("p (kt n) -> p kt n", kt=16),
                    wihT_in[:].rearrange("(kt p) n -> p kt n", p=128),
                )
                for m in range(MT):
                    xT = a3x.tile([128, 16 * 128], DT)
                    xT_v = xT[:].rearrange("p (kt n) -> p kt n", kt=16)
                    n0 = 128 * m
                    while n0 < 128 * (m + 1):
                        rk = n0 // TOK
                        lo = n0 - rk * TOK
                        hi = min(lo + (128 * (m + 1) - n0), TOK)
                        d0 = n0 - 128 * m
                        nc.sync.dma_start(
                            xT_v[:, :, d0:d0 + (hi - lo)],
                            XTf_v[:, rk, :, lo:hi],
                        )
                        n0 += hi - lo
                    ps0 = a3ps.tile([128, 512], f32)
                    ps1 = a3ps.tile([128, 512], f32)
                    for kt in range(16):
                        lhsT = xT[:, 128 * kt:128 * kt + 128]
                        nc.tensor.matmul(ps0[:], lhsT=(lhsT),
                                         rhs=(wih_sb[:, GC * kt:GC * kt + 512]),
                                         start=(kt == 0), stop=(kt == 15))
                        nc.tensor.matmul(ps1[:], lhsT=(lhsT),
                                         rhs=(wih_sb[:, GC * kt + 512:GC * (kt + 1)]),
                                         start=(kt == 0), stop=(kt == 15))
                    gxo = a3o.tile([128, GC], f32)
                    nc.vector.tensor_copy(gxo[:, 0:512], ps0[:])
                    nc.vector.tensor_copy(gxo[:, 512:GC], ps1[:])
                    nc.sync.dma_start(gx_d[128 * m:128 * (m + 1), :], gxo[:])

            # ---------------- Phase B: recurrence ----------------
            with tc.tile_pool(name="bc", bufs=1) as bc, \
                 tc.tile_pool(name="bh", bufs=2) as bh, \
                 tc.tile_pool(name="bgx", bufs=3) as bgx, \
                 tc.tile_pool(name="bel", bufs=2) as bel, \
                 tc.tile_pool(name="bcc", bufs=3) as bcell, \
                 tc.tile_pool(name="bo", bufs=3) as bo, \
                 tc.tile_pool(name="bpg", bufs=2, space="PSUM") as bpg, \
                 tc.tile_pool(name="bpt", bufs=1, space="PSUM") as bpt, \
                 tc.tile_pool(name="bpd", bufs=2, space="PSUM") as bpd:
                whh_sb = bc.tile([128, 16 * GC], DT)
                nc.sync.dma_start(
                    whh_sb[:].rearrange("p (kt n) -> p kt n", kt=16),
                    whhT_in[:].rearrange("(kt p) n -> p kt n", p=128),
                )
                dec_sb = bc.tile([128, 16 * S], DT)
                nc.sync.dma_start(
                    dec_sb[:].rearrange("p (kt n) -> p kt n", kt=16),
                    decWT_in[:].rearrange("(kt p) n -> p kt n", p=128),
                )
                bias_sb = bc.tile([B, GC], f32)
                bias_bc_ap = bass.AP(tensor=bias_in.ap().tensor, offset=0,
                                     ap=[[0, B], bias_in.ap().ap[1]])
                nc.sync.dma_start(bias_sb[:], bias_bc_ap)
                decb_sb = bc.tile([B, S], f32)
                decb_bc_ap = bass.AP(tensor=decb_in.ap().tensor, offset=0,
                                     ap=[[0, B], decb_in.ap().ap[1]])
                nc.sync.dma_start(decb_sb[:], decb_bc_ap)
                id32f = bc.tile([32, 32], f32)
                make_identity(nc, id32f[:])
                id32 = bc.tile([32, 32], DT)
                nc.vector.tensor_copy(id32[:], id32f[:])

                c_cur = bcell.tile([B, U], f32, tag="c")
                nc.sync.dma_start(c_cur[:], c0_in[:])
                hT_cur = bh.tile([128, 16 * B], DT, tag="hT")
                nc.sync.dma_start(
                    hT_cur[:].rearrange("p (kt n) -> p kt n", kt=16),
                    h0T_in[:].rearrange("(kt p) n -> p kt n", p=128),
                )

                for t in range(T):
                    gx_t = bgx.tile([B, GC], f32, tag="gx")
                    nc.sync.dma_start(gx_t[:], gx_d[B * t:B * (t + 1), :])
                    gxb = bgx.tile([B, GC], DT, tag="gxb")
                    nc.vector.tensor_add(gxb[:], gx_t[:], bias_sb[:])

                    ps = bpg.tile([B, GC], f32)
                    nc.tensor.matmul(ps[:, 0:512], lhsT=(id32[:]), rhs=(gxb[:, 0:512]),
                                     start=True, stop=False)
                    nc.tensor.matmul(ps[:, 512:1024], lhsT=(id32[:]), rhs=(gxb[:, 512:1024]),
                                     start=True, stop=False)
                    for kt in range(16):
                        lhsT = hT_cur[:, B * kt:B * (kt + 1)]
                        nc.tensor.matmul(ps[:, 0:512], lhsT=(lhsT),
                                         rhs=(whh_sb[:, GC * kt:GC * kt + 512]),
                                         start=False, stop=(kt == 15))
                        nc.tensor.matmul(ps[:, 512:1024], lhsT=(lhsT),
                                         rhs=(whh_sb[:, GC * kt + 512:GC * (kt + 1)]),
                                         start=False, stop=(kt == 15))

                    # gate columns: [i(0:256) | f(256:512) | o(512:768) | g(768:1024)]
                    sig = bel.tile([B, 768], f32, tag="sig")
                    nc.scalar.activation(sig[:], ps[:, 0:768],
                                         mybir.ActivationFunctionType.Sigmoid)
                    tng = bel.tile([B, U], f32, tag="tng")
                    nc.scalar.activation(tng[:], ps[:, 768:1024],
                                         mybir.ActivationFunctionType.Tanh)
                    t1 = bel.tile([B, U], f32, tag="t1")
                    nc.vector.tensor_mul(t1[:], sig[:, 0:256], tng[:])
                    t2 = bel.tile([B, U], f32, tag="t2")
                    nc.vector.tensor_mul(t2[:], sig[:, 256:512], c_cur[:])
                    c_new = bcell.tile([B, U], f32, tag="c")
                    nc.vector.tensor_add(c_new[:], t1[:], t2[:])
                    tnc = bel.tile([B, U], f32, tag="tnc")
                    nc.scalar.activation(tnc[:], c_new[:],
                                         mybir.ActivationFunctionType.Tanh)
                    h16 = bel.tile([B, U], DT, tag="h16")
                    nc.vector.tensor_mul(h16[:], sig[:, 512:768], tnc[:])

                    tp = bpt.tile([128, 64], DT)
                    nc.tensor.transpose((tp[:, 0:32]), (h16[:, 0:128]), (id32[:]))
                    nc.tensor.transpose((tp[:, 32:64]), (h16[:, 128:256]), (id32[:]))
                    hT_sb = bel.tile([128, 64], DT, tag="hTsb")
                    nc.vector.tensor_copy(hT_sb[:], tp[:])

                    hb = dramcc.tile([2 * 128, B], DT, tag="hb")
                    nc.sync.dma_start(
                        hb[:].rearrange("(a p) f -> p a f", p=128),
                        hT_sb[:].rearrange("p (a f) -> p a f", a=2),
                    )
                    hfull = dramcc.tile([N_CORES * 2 * 128, B], DT, tag="hfull")
                    nc.gpsimd.collective_compute(
                        "AllGather", mybir.AluOpType.bypass,
                        ins=[hb.opt()], outs=[hfull.opt()],
                        replica_groups=[list(range(N_CORES))],
                    )
                    hT_new = bh.tile([128, 16 * B], DT, tag="hT")
                    nc.sync.dma_start(
                        hT_new[:].rearrange("p (kt n) -> p kt n", kt=16),
                        hfull[:].rearrange("(kt p) n -> p kt n", p=128),
                    )

                    # decoder for step t (h_t just gathered), overlaps next AG
                    dps = bpd.tile([B, S], f32)
                    for kt in range(16):
                        nc.tensor.matmul(dps[:], lhsT=(hT_new[:, B * kt:B * (kt + 1)]),
                                         rhs=(dec_sb[:, S * kt:S * (kt + 1)]),
                                         start=(kt == 0), stop=(kt == 15))
                    outsb = bo.tile([B, S], f32, tag="outsb")
                    nc.vector.tensor_add(outsb[:], dps[:], decb_sb[:])
                    nc.sync.dma_start(out_d[B * t:B * (t + 1), :], outsb[:])

                    if t == T - 1:
                        h32 = bel.tile([B, U], f32, tag="h32")
                        nc.vector.tensor_mul(h32[:], sig[:, 512:768], tnc[:])
                        nc.sync.dma_start(hout_d[:], h32[:])
                        nc.sync.dma_start(cout_d[:], c_new[:])

                    c_cur = c_new
                    hT_cur = hT_new

    nc.compile()
    return nc


def _prep_inputs(input_1, input_2, h0, c0, routers_info,
                 m1_W, m1_b, m2_W, m2_b, enc_W, enc_b,
                 lstm_Wih, lstm_Whh, lstm_bih, lstm_bhh, dec_W, dec_b):
    T = input_1.shape[1]
    TPC = T // N_CORES
    TOK = TPC * B

    f32 = np.float32
    input_1 = np.asarray(input_1, f32)
    input_2 = np.asarray(input_2, f32)
    routers = np.asarray(routers_info).astype(np.int64)

    # folded expert table T4[s] = [[m1_W^T @ We_u], [m2_W^T @ We_v], [bias]]
    enc_W = np.asarray(enc_W, f32)      # [S, 20, H]
    enc_b = np.asarray(enc_b, f32)      # [S, H]
    We_u = enc_W[:, 0:10, :]            # [S, 10, H]
    We_v = enc_W[:, 10:20, :]
    m1_W = np.asarray(m1_W, f32)        # [10, 2]
    m2_W = np.asarray(m2_W, f32)        # [10, 1]
    T4 = np.empty((S, 4, H), f32)
    T4[:, 0:2, :] = np.einsum("fr,sfh->srh", m1_W, We_u)
    T4[:, 2:3, :] = np.einsum("fr,sfh->srh", m2_W, We_v)
    T4[:, 3, :] = (np.einsum("f,sfh->sh", np.asarray(m1_b, f32), We_u)
                   + np.einsum("f,sfh->sh", np.asarray(m2_b, f32), We_v)
                   + enc_b)
    expt4 = np.ascontiguousarray(T4.reshape(S * 4, H)).astype(NPDT)

    # per-core gate-row order [i|f|o|g] for this core's 256 units
    Wih = np.asarray(lstm_Wih, f32)
    Whh = np.asarray(lstm_Whh, f32)
    bihh = np.asarray(lstm_bih, f32) + np.asarray(lstm_bhh, f32)
    dec_WT = np.ascontiguousarray(np.asarray(dec_W, f32).T).astype(NPDT)  # [H, S]
    decb = np.asarray(dec_b, f32).reshape(1, S)

    h0 = np.asarray(h0, f32)
    c0 = np.asarray(c0, f32)
    h0T = np.ascontiguousarray(h0.T).astype(NPDT)  # [H, B]

    # f4[n] = [x1_0, x1_1, x2_0, 1]
    x1 = input_1  # [B, T, 2]
    x2 = input_2  # [B, T, 1]

    in_maps = []
    for k in range(N_CORES):
        uk = np.arange(U * k, U * (k + 1))
        rows = np.concatenate([uk, H + uk, 3 * H + uk, 2 * H + uk])  # i,f,o,g
        wihT = np.ascontiguousarray(Wih[rows].T).astype(NPDT)  # [H, GC]
        whhT = np.ascontiguousarray(Whh[rows].T).astype(NPDT)
        biask = np.ascontiguousarray(bihh[rows].reshape(1, GC))

        ts0 = TPC * k
        # block-diag fused inputs: bd[4j+r, 32g+j] = f4[b=j, t=ts0+g, r]
        bd = np.zeros((128, TOK), f32)
        offs = np.zeros((128, TPC), np.int32)
        for g in range(TPC):
            t = ts0 + g
            for j in range(B):
                bd[4 * j + 0, B * g + j] = x1[j, t, 0]
                bd[4 * j + 1, B * g + j] = x1[j, t, 1]
                bd[4 * j + 2, B * g + j] = x2[j, t, 0]
                bd[4 * j + 3, B * g + j] = 1.0
                s = int(routers[j, t])
                offs[4 * j + 0, g] = 4 * s + 0
                offs[4 * j + 1, g] = 4 * s + 1
                offs[4 * j + 2, g] = 4 * s + 2
                offs[4 * j + 3, g] = 4 * s + 3

        in_maps.append({
            "expt4": expt4,
            "bd": bd.astype(NPDT),
            "offs": offs,
            "wihT": wihT,
            "whhT": whhT,
            "biask": biask,
            "decWT": dec_WT,
            "decb": decb,
            "h0T": h0T,
            "c0k": np.ascontiguousarray(c0[:, uk]),
        })
    return in_maps, T


def kernel(**inputs):
    in_maps, T = _prep_inputs(**inputs)
    if T not in _BUILD_CACHE:
        _BUILD_CACHE[T] = _build(T)
    nc = _BUILD_CACHE[T]

    res = run_bass_kernel_spmd(nc, in_maps, core_ids=list(range(N_CORES)))
    kernel.last_results = res

    out_tb = res.results[0]["out"]            # [T*B, S], rows t-major
    NTOK = T * B
    output = np.ascontiguousarray(
        out_tb.reshape(T, B, S).transpose(1, 0, 2).reshape(NTOK, S))
    hT = np.concatenate([res.results[k]["h_out"] for k in range(N_CORES)], axis=1)
    cT = np.concatenate([res.results[k]["c_out"] for k in range(N_CORES)], axis=1)
    return output.astype(np.float32), hT.astype(np.float32), cT.astype(np.float32)


# revision 8
# speedup vs baseline: 1.0343x; 1.0343x over previous
"""DeepKnowledgeTracing (LSTM + per-step MoE expert routing) Trainium2 kernel.

Strategy (8 NeuronCores, tensor-parallel over the 4H gate dimension):
  - Host folds the two tiny input Linears (2->10, 1->10) into the per-expert
    encoder table algebraically:  x[n] = [x1_0, x1_1, x2_0, 1] @ T4[skill[n]]
    with T4[s] = [[m1_W^T @ We_u[s]], [m2_W^T @ We_v[s]], [bias row]]  (exact).
  - Phase A1 (device): per-core indirect-DMA gather of T4 rows for its 1/8
    token shard + block-diagonal batched matmul -> X^T slice [H, 800].
  - Phase A2: AllGather X^T across the 8 cores.
  - Phase A3: gates_x = X @ Wih_k^T for this core's 1024 gate rows, all 6400
    tokens (the state-independent half of the LSTM FLOPs, fully parallel).
  - Phase B: 200-step recurrence. Core k owns 256 h-units (gate rows ordered
    [i|f|o|g]); per step: 16x2 matmuls vs resident Whh_k^T, fused elementwise
    on ACT/DVE, PE-transpose of h_new, 16KB AllGather of h, decoder matmul
    for the step overlapped with the collective.
  - Outputs: full logits [B*T, S] (every core computes them; core 0's used),
    per-core fp32 h/c finals assembled host-side.
"""

import os
import numpy as np
import ml_dtypes

import concourse.bass as bass
import concourse.bacc as bacc
import concourse.mybir as mybir
import concourse.tile as tile
from concourse.bass import IndirectOffsetOnAxis
from concourse.bass_utils import run_bass_kernel_spmd
from concourse.masks import make_identity

N_CORES = 8
B = 32
H = 2048
S = 256
GC = 4 * H // N_CORES   # 1024 gate columns per core (order [i|f|o|g] x 256 units)
U = H // N_CORES        # 256 hidden units per core

_DT_NAME = os.environ.get("DKT_DT", "float32r")
USE_F32R = _DT_NAME == "float32r"
DT = getattr(mybir.dt, _DT_NAME)
NPDT = ml_dtypes.bfloat16 if _DT_NAME == "bfloat16" else np.float32

_BUILD_CACHE = {}


def _build(T):
    TPC = T // N_CORES          # steps handled per core in phase A1
    TOK = TPC * B               # tokens per core in phase A1
    NTOK = T * B
    MT = NTOK // 128            # number of 128-token M-tiles in phase A3
    f32 = mybir.dt.float32

    nc = bacc.Bacc("TRN2", target_bir_lowering=False, debug=False,
                   num_devices=N_CORES)

    # ---- kernel I/O ----
    expt4 = nc.dram_tensor("expt4", [S * 4, H], DT, kind="ExternalInput")
    bd_in = nc.dram_tensor("bd", [128, TOK], DT, kind="ExternalInput")
    offs_in = nc.dram_tensor("offs", [128, TPC], mybir.dt.int32, kind="ExternalInput")
    wihT_in = nc.dram_tensor("wihT", [H, GC], DT, kind="ExternalInput")
    whhT_in = nc.dram_tensor("whhT", [H, GC], DT, kind="ExternalInput")
    bias_in = nc.dram_tensor("biask", [1, GC], f32, kind="ExternalInput")
    decWT_in = nc.dram_tensor("decWT", [H, S], DT, kind="ExternalInput")
    decb_in = nc.dram_tensor("decb", [1, S], f32, kind="ExternalInput")
    h0T_in = nc.dram_tensor("h0T", [H, B], DT, kind="ExternalInput")
    c0_in = nc.dram_tensor("c0k", [B, U], f32, kind="ExternalInput")

    out_d = nc.dram_tensor("out", [NTOK, S], f32, kind="ExternalOutput")
    hout_d = nc.dram_tensor("h_out", [B, U], f32, kind="ExternalOutput")
    cout_d = nc.dram_tensor("c_out", [B, U], f32, kind="ExternalOutput")

    with tile.TileContext(nc) as tc:
        with tc.tile_pool(name="dram", bufs=1, space="DRAM") as dramp, \
             tc.tile_pool(name="dramcc", bufs=4, space="DRAM") as dramcc:
            XT_d = dramp.tile([H, TOK], DT)
            XTfull_d = dramp.tile([N_CORES * H, TOK], DT)
            gx_d = dramp.tile([NTOK, GC], DT)

            # ---------------- Phase A1: expert encode -> X^T ----------------
            with tc.tile_pool(name="a1sb", bufs=1) as a1c, \
                 tc.tile_pool(name="a1g", bufs=3) as a1g, \
                 tc.tile_pool(name="a1ps", bufs=2, space="PSUM") as a1ps:
                bd_sb = a1c.tile([128, TOK], DT)
                nc.sync.dma_start(bd_sb[:], bd_in[:])
                offs_sb = a1c.tile([128, TPC], mybir.dt.int32)
                nc.sync.dma_start(offs_sb[:], offs_in[:])
                xt_stage = a1c.tile([128, 16 * TOK], DT)
                xt_v = xt_stage[:].rearrange("p (pt n) -> p pt n", pt=16)

                for g in range(TPC):
                    gath = a1g.tile([128, H], DT)
                    nc.gpsimd.indirect_dma_start(
                        out=gath[:], out_offset=None,
                        in_=expt4[:],
                        in_offset=IndirectOffsetOnAxis(ap=offs_sb[:, g:g + 1], axis=0),
                    )
                    ps = a1ps.tile([128, 512], f32)
                    for pt in range(16):
                        nc.tensor.matmul(
                            ps[:, 32 * pt:32 * pt + 32],
                            lhsT=(gath[:, 128 * pt:128 * pt + 128]),
                            rhs=(bd_sb[:, 32 * g:32 * g + 32]),
                            start=True, stop=True,
                        )
                    nc.vector.tensor_copy(
                        xt_v[:, :, 32 * g:32 * g + 32],
                        ps[:].rearrange("p (pt j) -> p pt j", pt=16),
                    )
                nc.sync.dma_start(
                    XT_d[:].rearrange("(pt p) n -> p pt n", p=128), xt_stage[:].rearrange("p (pt n) -> p pt n", pt=16)
                )

            # ---------------- Phase A2: AllGather X^T ----------------
            nc.gpsimd.collective_compute(
                "AllGather", mybir.AluOpType.bypass,
                ins=[XT_d.opt()], outs=[XTfull_d.opt()],
                replica_groups=[list(range(N_CORES))],
            )
            XTf_v = XTfull_d[:].rearrange("(rk pt p) n -> p rk pt n", p=128, rk=N_CORES)

            # ---------------- Phase A3: gates_x = X @ WihT_k ----------------
            with tc.tile_pool(name="a3c", bufs=1) as a3c, \
                 tc.tile_pool(name="a3x", bufs=3) as a3x, \
                 tc.tile_pool(name="a3o", bufs=3) as a3o, \
                 tc.tile_pool(name="a3ps", bufs=2, space="PSUM") as a3ps:
                wih_sb = a3c.tile([128, 16 * GC], DT)
                bias3_sb = a3c.tile([128, GC], f32)
                bias3_ap = bass.AP(tensor=bias_in.ap().tensor, offset=0,
                                   ap=[[0, 128], bias_in.ap().ap[1]])
                nc.sync.dma_start(bias3_sb[:], bias3_ap)
                nc.sync.dma_start(
                    wih_sb[:].rearrange("p (kt n) -> p kt n", kt=16),
                    wihT_in[:].rearrange("(kt p) n -> p kt n", p=128),
                )
                for m in range(MT):
                    xT = a3x.tile([128, 16 * 128], DT)
                    xT_v = xT[:].rearrange("p (kt n) -> p kt n", kt=16)
                    n0 = 128 * m
                    while n0 < 128 * (m + 1):
                        rk = n0 // TOK
                        lo = n0 - rk * TOK
                        hi = min(lo + (128 * (m + 1) - n0), TOK)
                        d0 = n0 - 128 * m
                        nc.sync.dma_start(
                            xT_v[:, :, d0:d0 + (hi - lo)],
                            XTf_v[:, rk, :, lo:hi],
                        )
                        n0 += hi - lo
                    ps0 = a3ps.tile([128, 512], f32)
                    ps1 = a3ps.tile([128, 512], f32)
                    for kt in range(16):
                        lhsT = xT[:, 128 * kt:128 * kt + 128]
                        nc.tensor.matmul(ps0[:], lhsT=(lhsT),
                                         rhs=(wih_sb[:, GC * kt:GC * kt + 512]),
                                         start=(kt == 0), stop=(kt == 15))
                        nc.tensor.matmul(ps1[:], lhsT=(lhsT),
                                         rhs=(wih_sb[:, GC * kt + 512:GC * (kt + 1)]),
                                         start=(kt == 0), stop=(kt == 15))
                    gxo = a3o.tile([128, GC], DT)
                    nc.vector.tensor_add(gxo[:, 0:512], ps0[:], bias3_sb[:, 0:512])
                    nc.vector.tensor_add(gxo[:, 512:GC], ps1[:], bias3_sb[:, 512:GC])
                    nc.sync.dma_start(gx_d[128 * m:128 * (m + 1), :], gxo[:])

            # ---------------- Phase B: recurrence ----------------
            with tc.tile_pool(name="bc", bufs=1) as bc, \
                 tc.tile_pool(name="bh", bufs=2) as bh, \
                 tc.tile_pool(name="bgx", bufs=3) as bgx, \
                 tc.tile_pool(name="bel", bufs=2) as bel, \
                 tc.tile_pool(name="bcc", bufs=3) as bcell, \
                 tc.tile_pool(name="bo", bufs=3) as bo, \
                 tc.tile_pool(name="bpg", bufs=2, space="PSUM") as bpg, \
                 tc.tile_pool(name="bpt", bufs=1, space="PSUM") as bpt, \
                 tc.tile_pool(name="bpd", bufs=2, space="PSUM") as bpd, \
                 tc.tile_pool(name="bpw", bufs=1, space="PSUM") as bpw:
                whh_sb = bc.tile([128, 16 * GC], DT)
                nc.sync.dma_start(
                    whh_sb[:].rearrange("p (kt n) -> p kt n", kt=16),
                    whhT_in[:].rearrange("(kt p) n -> p kt n", p=128),
                )
                dec_sb = bc.tile([128, 16 * S], DT)
                nc.sync.dma_start(
                    dec_sb[:].rearrange("p (kt n) -> p kt n", kt=16),
                    decWT_in[:].rearrange("(kt p) n -> p kt n", p=128),
                )
                decb_sb = bc.tile([B, S], f32)
                decb_bc_ap = bass.AP(tensor=decb_in.ap().tensor, offset=0,
                                     ap=[[0, B], decb_in.ap().ap[1]])
                nc.sync.dma_start(decb_sb[:], decb_bc_ap)
                NWARM = int(os.environ.get("DKT_WARM", "12"))
                id32f = bc.tile([32, 32], f32)
                make_identity(nc, id32f[:])
                id32 = bc.tile([32, 32], DT)
                nc.vector.tensor_copy(id32[:], id32f[:])

                c_cur = bcell.tile([B, U], f32, tag="c")
                nc.sync.dma_start(c_cur[:], c0_in[:])
                hT_cur = bh.tile([128, 16 * B], DT, tag="hT")
                nc.sync.dma_start(
                    hT_cur[:].rearrange("p (kt n) -> p kt n", kt=16),
                    h0T_in[:].rearrange("(kt p) n -> p kt n", p=128),
                )

                for t in range(T):
                    gx_t = bgx.tile([B, GC], DT, tag="gx")
                    nc.sync.dma_start(gx_t[:], gx_d[B * t:B * (t + 1), :])

                    ps = bpg.tile([B, GC], f32)
                    nc.tensor.matmul(ps[:, 0:512], lhsT=(id32[:]), rhs=(gx_t[:, 0:512]),
                                     start=True, stop=False)
                    nc.tensor.matmul(ps[:, 512:1024], lhsT=(id32[:]), rhs=(gx_t[:, 512:1024]),
                                     start=True, stop=False)
                    for kt in range(16):
                        lhsT = hT_cur[:, B * kt:B * (kt + 1)]
                        nc.tensor.matmul(ps[:, 0:512], lhsT=(lhsT),
                                         rhs=(whh_sb[:, GC * kt:GC * kt + 512]),
                                         start=False, stop=(kt == 15))
                        nc.tensor.matmul(ps[:, 512:1024], lhsT=(lhsT),
                                         rhs=(whh_sb[:, GC * kt + 512:GC * (kt + 1)]),
                                         start=False, stop=(kt == 15))

                    # gate columns: [i(0:256) | f(256:512) | o(512:768) | g(768:1024)]
                    sig = bel.tile([B, 768], f32, tag="sig")
                    nc.scalar.activation(sig[:, 0:512], ps[:, 0:512],
                                         mybir.ActivationFunctionType.Sigmoid)
                    tng = bel.tile([B, U], f32, tag="tng")
                    nc.scalar.activation(tng[:], ps[:, 768:1024],
                                         mybir.ActivationFunctionType.Tanh)
                    nc.scalar.activation(sig[:, 512:768], ps[:, 512:768],
                                         mybir.ActivationFunctionType.Sigmoid)
                    t1 = bel.tile([B, U], f32, tag="t1")
                    nc.vector.tensor_mul(t1[:], sig[:, 0:256], tng[:])
                    t2 = bel.tile([B, U], f32, tag="t2")
                    nc.vector.tensor_mul(t2[:], sig[:, 256:512], c_cur[:])
                    c_new = bcell.tile([B, U], f32, tag="c")
                    nc.vector.tensor_add(c_new[:], t1[:], t2[:])
                    tnc = bel.tile([B, U], f32, tag="tnc")
                    nc.scalar.activation(tnc[:], c_new[:],
                                         mybir.ActivationFunctionType.Tanh)
                    h16 = bel.tile([B, U], DT, tag="h16")
                    nc.vector.tensor_mul(h16[:], sig[:, 512:768], tnc[:])

                    tp = bpt.tile([128, 64], DT)
                    nc.tensor.transpose((tp[:, 0:32]), (h16[:, 0:128]), (id32[:]))
                    nc.tensor.transpose((tp[:, 32:64]), (h16[:, 128:256]), (id32[:]))
                    hT_sb = bel.tile([128, 64], DT, tag="hTsb")
                    nc.vector.tensor_copy(hT_sb[:], tp[:])

                    hb = dramcc.tile([2 * 128, B], DT, tag="hb")
                    nc.sync.dma_start(
                        hb[:].rearrange("(a p) f -> p a f", p=128),
                        hT_sb[:].rearrange("p (a f) -> p a f", a=2),
                    )
                    hfull = dramcc.tile([N_CORES * 2 * 128, B], DT, tag="hfull")
                    nc.gpsimd.collective_compute(
                        "AllGather", mybir.AluOpType.bypass,
                        ins=[hb.opt()], outs=[hfull.opt()],
                        replica_groups=[list(range(N_CORES))],
                    )
                    hT_new = bh.tile([128, 16 * B], DT, tag="hT")
                    hTn_v = hT_new[:].rearrange("p (kt n) -> p kt n", kt=16)
                    hf_v = hfull[:].rearrange("(kt p) n -> p kt n", p=128)
                    nc.sync.dma_start(hTn_v[:, 0:8], hf_v[:, 0:8])
                    nc.sync.dma_start(hTn_v[:, 8:16], hf_v[:, 8:16])

                    # decoder for step t (h_t just gathered), overlaps next AG
                    dps = bpd.tile([B, S], f32)
                    for kt in range(16):
                        nc.tensor.matmul(dps[:], lhsT=(hT_new[:, B * kt:B * (kt + 1)]),
                                         rhs=(dec_sb[:, S * kt:S * (kt + 1)]),
                                         start=(kt == 0), stop=(kt == 15))
                    outsb = bo.tile([B, S], f32, tag="outsb")
                    nc.vector.tensor_add(outsb[:], dps[:], decb_sb[:])
                    nc.sync.dma_start(out_d[B * t:B * (t + 1), :], outsb[:])

                    if t == T - 1:
                        h32 = bel.tile([B, U], f32, tag="h32")
                        nc.vector.tensor_mul(h32[:], sig[:, 512:768], tnc[:])
                        nc.sync.dma_start(hout_d[:], h32[:])
                        nc.sync.dma_start(cout_d[:], c_new[:])

                    if NWARM and t < T - 1:
                        wps = bpw.tile([B, 256], f32)
                        for w in range(NWARM):
                            nc.tensor.matmul(wps[:], lhsT=id32f[:],
                                             rhs=decb_sb[:, 0:256],
                                             start=True, stop=True,
                                             skip_group_check=True)

                    c_cur = c_new
                    hT_cur = hT_new

    nc.compile()
    return nc


def _prep_inputs(input_1, input_2, h0, c0, routers_info,
                 m1_W, m1_b, m2_W, m2_b, enc_W, enc_b,
                 lstm_Wih, lstm_Whh, lstm_bih, lstm_bhh, dec_W, dec_b):
    T = input_1.shape[1]
    TPC = T // N_CORES
    TOK = TPC * B

    f32 = np.float32
    input_1 = np.asarray(input_1, f32)
    input_2 = np.asarray(input_2, f32)
    routers = np.asarray(routers_info).astype(np.int64)

    # folded expert table T4[s] = [[m1_W^T @ We_u], [m2_W^T @ We_v], [bias]]
    enc_W = np.asarray(enc_W, f32)      # [S, 20, H]
    enc_b = np.asarray(enc_b, f32)      # [S, H]
    We_u = enc_W[:, 0:10, :]            # [S, 10, H]
    We_v = enc_W[:, 10:20, :]
    m1_W = np.asarray(m1_W, f32)        # [10, 2]
    m2_W = np.asarray(m2_W, f32)        # [10, 1]
    T4 = np.empty((S, 4, H), f32)
    T4[:, 0:2, :] = np.einsum("fr,sfh->srh", m1_W, We_u)
    T4[:, 2:3, :] = np.einsum("fr,sfh->srh", m2_W, We_v)
    T4[:, 3, :] = (np.einsum("f,sfh->sh", np.asarray(m1_b, f32), We_u)
                   + np.einsum("f,sfh->sh", np.asarray(m2_b, f32), We_v)
                   + enc_b)
    expt4 = np.ascontiguousarray(T4.reshape(S * 4, H)).astype(NPDT)

    # per-core gate-row order [i|f|o|g] for this core's 256 units
    Wih = np.asarray(lstm_Wih, f32)
    Whh = np.asarray(lstm_Whh, f32)
    bihh = np.asarray(lstm_bih, f32) + np.asarray(lstm_bhh, f32)
    dec_WT = np.ascontiguousarray(np.asarray(dec_W, f32).T).astype(NPDT)  # [H, S]
    decb = np.asarray(dec_b, f32).reshape(1, S)

    h0 = np.asarray(h0, f32)
    c0 = np.asarray(c0, f32)
    h0T = np.ascontiguousarray(h0.T).astype(NPDT)  # [H, B]

    # f4[n] = [x1_0, x1_1, x2_0, 1]
    x1 = input_1  # [B, T, 2]
    x2 = input_2  # [B, T, 1]

    in_maps = []
    for k in range(N_CORES):
        uk = np.arange(U * k, U * (k + 1))
        rows = np.concatenate([uk, H + uk, 3 * H + uk, 2 * H + uk])  # i,f,o,g
        wihT = np.ascontiguousarray(Wih[rows].T).astype(NPDT)  # [H, GC]
        whhT = np.ascontiguousarray(Whh[rows].T).astype(NPDT)
        biask = np.ascontiguousarray(bihh[rows].reshape(1, GC))

        ts0 = TPC * k
        # block-diag fused inputs: bd[4j+r, 32g+j] = f4[b=j, t=ts0+g, r]
        bd = np.zeros((128, TOK), f32)
        offs = np.zeros((128, TPC), np.int32)
        for g in range(TPC):
            t = ts0 + g
            for j in range(B):
                bd[4 * j + 0, B * g + j] = x1[j, t, 0]
                bd[4 * j + 1, B * g + j] = x1[j, t, 1]
                bd[4 * j + 2, B * g + j] = x2[j, t, 0]
                bd[4 * j + 3, B * g + j] = 1.0
                s = int(routers[j, t])
                offs[4 * j + 0, g] = 4 * s + 0
                offs[4 * j + 1, g] = 4 * s + 1
                offs[4 * j + 2, g] = 4 * s + 2
                offs[4 * j + 3, g] = 4 * s + 3

        in_maps.append({
            "expt4": expt4,
            "bd": bd.astype(NPDT),
            "offs": offs,
            "wihT": wihT,
            "whhT": whhT,
            "biask": biask,
            "decWT": dec_WT,
            "decb": decb,
            "h0T": h0T,
            "c0k": np.ascontiguousarray(c0[:, uk]),
        })
    return in_maps, T


def kernel(**inputs):
    in_maps, T = _prep_inputs(**inputs)
    if T not in _BUILD_CACHE:
        _BUILD_CACHE[T] = _build(T)
    nc = _BUILD_CACHE[T]

    res = run_bass_kernel_spmd(nc, in_maps, core_ids=list(range(N_CORES)))
    kernel.last_results = res

    out_tb = res.results[0]["out"]            # [T*B, S], rows t-major
    NTOK = T * B
    output = np.ascontiguousarray(
        out_tb.reshape(T, B, S).transpose(1, 0, 2).reshape(NTOK, S))
    hT = np.concatenate([res.results[k]["h_out"] for k in range(N_CORES)], axis=1)
    cT = np.concatenate([res.results[k]["c_out"] for k in range(N_CORES)], axis=1)
    return output.astype(np.float32), hT.astype(np.float32), cT.astype(np.float32)


# revision 12
# speedup vs baseline: 1.0531x; 1.0182x over previous
"""DeepKnowledgeTracing (LSTM + per-step MoE expert routing) Trainium2 kernel.

Strategy (8 NeuronCores, tensor-parallel over the 4H gate dimension):
  - Host folds the two tiny input Linears (2->10, 1->10) into the per-expert
    encoder table algebraically:  x[n] = [x1_0, x1_1, x2_0, 1] @ T4[skill[n]]
    with T4[s] = [[m1_W^T @ We_u[s]], [m2_W^T @ We_v[s]], [bias row]]  (exact).
  - Phase A1 (device): per-core indirect-DMA gather of T4 rows for its 1/8
    token shard + block-diagonal batched matmul -> X^T slice [H, 800].
  - Phase A2: AllGather X^T across the 8 cores.
  - Phase A3: gates_x = X @ Wih_k^T for this core's 1024 gate rows, all 6400
    tokens (the state-independent half of the LSTM FLOPs, fully parallel).
  - Phase B: 200-step recurrence. Core k owns 256 h-units (gate rows ordered
    [i|f|o|g]); per step: 16x2 matmuls vs resident Whh_k^T, fused elementwise
    on ACT/DVE, PE-transpose of h_new, 16KB AllGather of h, decoder matmul
    for the step overlapped with the collective.
  - Outputs: full logits [B*T, S] (every core computes them; core 0's used),
    per-core fp32 h/c finals assembled host-side.
"""

import os
import numpy as np
import ml_dtypes

import concourse.bass as bass
import concourse.bacc as bacc
import concourse.mybir as mybir
import concourse.tile as tile
from concourse.bass import IndirectOffsetOnAxis
from concourse.bass_utils import run_bass_kernel_spmd
from concourse.masks import make_identity

N_CORES = 8
B = 32
H = 2048
S = 256
GC = 4 * H // N_CORES   # 1024 gate columns per core (order [i|f|o|g] x 256 units)
U = H // N_CORES        # 256 hidden units per core

_DT_NAME = os.environ.get("DKT_DT", "float32r")
USE_F32R = _DT_NAME == "float32r"
DT = getattr(mybir.dt, _DT_NAME)
NPDT = ml_dtypes.bfloat16 if _DT_NAME == "bfloat16" else np.float32
# A-phase (expert encode + gates_x) matmul dtype: bf16 streams 2x vs f32r
_ADT_NAME = os.environ.get("DKT_ADT", "bfloat16")
ADT = getattr(mybir.dt, _ADT_NAME)
NPADT = ml_dtypes.bfloat16 if _ADT_NAME == "bfloat16" else np.float32

_BUILD_CACHE = {}


def _build(T):
    TPC = T // N_CORES          # steps handled per core in phase A1
    TOK = TPC * B               # tokens per core in phase A1
    NTOK = T * B
    MT = NTOK // 128            # number of 128-token M-tiles in phase A3
    f32 = mybir.dt.float32

    nc = bacc.Bacc("TRN2", target_bir_lowering=False, debug=False,
                   num_devices=N_CORES)

    # ---- kernel I/O ----
    expt4 = nc.dram_tensor("expt4", [S * 4, H], ADT, kind="ExternalInput")
    bd_in = nc.dram_tensor("bd", [128, TOK], ADT, kind="ExternalInput")
    offs_in = nc.dram_tensor("offs", [128, TPC], mybir.dt.int32, kind="ExternalInput")
    wihT_in = nc.dram_tensor("wihT", [H, GC], ADT, kind="ExternalInput")
    whhT_in = nc.dram_tensor("whhT", [H, GC], DT, kind="ExternalInput")
    bias_in = nc.dram_tensor("biask", [1, GC], f32, kind="ExternalInput")
    decWT_in = nc.dram_tensor("decWT", [H, S], DT, kind="ExternalInput")
    decb_in = nc.dram_tensor("decb", [1, S], f32, kind="ExternalInput")
    h0T_in = nc.dram_tensor("h0T", [H, B], DT, kind="ExternalInput")
    c0_in = nc.dram_tensor("c0k", [B, U], f32, kind="ExternalInput")

    out_d = nc.dram_tensor("out", [NTOK, S], f32, kind="ExternalOutput")
    hout_d = nc.dram_tensor("h_out", [B, U], f32, kind="ExternalOutput")
    cout_d = nc.dram_tensor("c_out", [B, U], f32, kind="ExternalOutput")

    with tile.TileContext(nc) as tc:
        with tc.tile_pool(name="dram", bufs=1, space="DRAM") as dramp, \
             tc.tile_pool(name="dramcc", bufs=4, space="DRAM") as dramcc:
            XT_d = dramp.tile([H, TOK], ADT)
            XTfull_d = dramp.tile([N_CORES * H, TOK], ADT)
            gx_d = dramp.tile([NTOK, GC], DT)

            # ---------------- Phase A1: expert encode -> X^T ----------------
            with tc.tile_pool(name="a1sb", bufs=1) as a1c, \
                 tc.tile_pool(name="a1g", bufs=3) as a1g, \
                 tc.tile_pool(name="a1ps", bufs=2, space="PSUM") as a1ps:
                bd_sb = a1c.tile([128, TOK], ADT)
                nc.sync.dma_start(bd_sb[:], bd_in[:])
                offs_sb = a1c.tile([128, TPC], mybir.dt.int32)
                nc.sync.dma_start(offs_sb[:], offs_in[:])
                xt_stage = a1c.tile([128, 16 * TOK], ADT)
                xt_v = xt_stage[:].rearrange("p (pt n) -> p pt n", pt=16)

                for g in range(TPC):
                    gath = a1g.tile([128, H], ADT)
                    nc.gpsimd.indirect_dma_start(
                        out=gath[:], out_offset=None,
                        in_=expt4[:],
                        in_offset=IndirectOffsetOnAxis(ap=offs_sb[:, g:g + 1], axis=0),
                    )
                    ps = a1ps.tile([128, 512], f32)
                    for pt in range(16):
                        nc.tensor.matmul(
                            ps[:, 32 * pt:32 * pt + 32],
                            lhsT=(gath[:, 128 * pt:128 * pt + 128]),
                            rhs=(bd_sb[:, 32 * g:32 * g + 32]),
                            start=True, stop=True,
                        )
                    nc.vector.tensor_copy(
                        xt_v[:, :, 32 * g:32 * g + 32],
                        ps[:].rearrange("p (pt j) -> p pt j", pt=16),
                    )
                nc.sync.dma_start(
                    XT_d[:].rearrange("(pt p) n -> p pt n", p=128), xt_stage[:].rearrange("p (pt n) -> p pt n", pt=16)
                )

            # ---------------- Phase A2: AllGather X^T ----------------
            nc.gpsimd.collective_compute(
                "AllGather", mybir.AluOpType.bypass,
                ins=[XT_d.opt()], outs=[XTfull_d.opt()],
                replica_groups=[list(range(N_CORES))],
            )
            XTf_v = XTfull_d[:].rearrange("(rk pt p) n -> p rk pt n", p=128, rk=N_CORES)

            # ---------------- Phase A3: gates_x = X @ WihT_k ----------------
            with tc.tile_pool(name="a3c", bufs=1) as a3c, \
                 tc.tile_pool(name="a3x", bufs=3) as a3x, \
                 tc.tile_pool(name="a3o", bufs=3) as a3o, \
                 tc.tile_pool(name="a3ps", bufs=2, space="PSUM") as a3ps:
                wih_sb = a3c.tile([128, 16 * GC], ADT)
                bias3_sb = a3c.tile([128, GC], f32)
                bias3_ap = bass.AP(tensor=bias_in.ap().tensor, offset=0,
                                   ap=[[0, 128], bias_in.ap().ap[1]])
                nc.sync.dma_start(bias3_sb[:], bias3_ap)
                nc.sync.dma_start(
                    wih_sb[:].rearrange("p (kt n) -> p kt n", kt=16),
                    wihT_in[:].rearrange("(kt p) n -> p kt n", p=128),
                )
                for m in range(MT):
                    xT = a3x.tile([128, 16 * 128], ADT)
                    xT_v = xT[:].rearrange("p (kt n) -> p kt n", kt=16)
                    n0 = 128 * m
                    while n0 < 128 * (m + 1):
                        rk = n0 // TOK
                        lo = n0 - rk * TOK
                        hi = min(lo + (128 * (m + 1) - n0), TOK)
                        d0 = n0 - 128 * m
                        nc.sync.dma_start(
                            xT_v[:, :, d0:d0 + (hi - lo)],
                            XTf_v[:, rk, :, lo:hi],
                        )
                        n0 += hi - lo
                    ps0 = a3ps.tile([128, 512], f32)
                    ps1 = a3ps.tile([128, 512], f32)
                    for kt in range(16):
                        lhsT = xT[:, 128 * kt:128 * kt + 128]
                        nc.tensor.matmul(ps0[:], lhsT=(lhsT),
                                         rhs=(wih_sb[:, GC * kt:GC * kt + 512]),
                                         start=(kt == 0), stop=(kt == 15))
                        nc.tensor.matmul(ps1[:], lhsT=(lhsT),
                                         rhs=(wih_sb[:, GC * kt + 512:GC * (kt + 1)]),
                                         start=(kt == 0), stop=(kt == 15))
                    gxo = a3o.tile([128, GC], DT)
                    nc.vector.tensor_add(gxo[:, 0:512], ps0[:], bias3_sb[:, 0:512])
                    nc.vector.tensor_add(gxo[:, 512:GC], ps1[:], bias3_sb[:, 512:GC])
                    nc.sync.dma_start(gx_d[128 * m:128 * (m + 1), :], gxo[:])

            # ---------------- Phase B: recurrence ----------------
            with tc.tile_pool(name="bc", bufs=1) as bc, \
                 tc.tile_pool(name="bh", bufs=2) as bh, \
                 tc.tile_pool(name="bgx", bufs=3) as bgx, \
                 tc.tile_pool(name="bel", bufs=2) as bel, \
                 tc.tile_pool(name="bcc", bufs=3) as bcell, \
                 tc.tile_pool(name="bo", bufs=3) as bo, \
                 tc.tile_pool(name="bpg", bufs=2, space="PSUM") as bpg, \
                 tc.tile_pool(name="bpt", bufs=1, space="PSUM") as bpt, \
                 tc.tile_pool(name="bpd", bufs=1, space="PSUM") as bpd:
                whh_sb = bc.tile([128, 16 * GC], DT)
                nc.sync.dma_start(
                    whh_sb[:].rearrange("p (kt n) -> p kt n", kt=16),
                    whhT_in[:].rearrange("(kt p) n -> p kt n", p=128),
                )
                dec_sb = bc.tile([128, 16 * S], DT)
                nc.sync.dma_start(
                    dec_sb[:].rearrange("p (kt n) -> p kt n", kt=16),
                    decWT_in[:].rearrange("(kt p) n -> p kt n", p=128),
                )
                decb_sb = bc.tile([B, S], f32)
                decb_bc_ap = bass.AP(tensor=decb_in.ap().tensor, offset=0,
                                     ap=[[0, B], decb_in.ap().ap[1]])
                nc.sync.dma_start(decb_sb[:], decb_bc_ap)
                id32f = bc.tile([32, 32], f32)
                make_identity(nc, id32f[:])
                id32 = bc.tile([32, 32], DT)
                nc.vector.tensor_copy(id32[:], id32f[:])

                c_cur = bcell.tile([B, U], f32, tag="c")
                nc.sync.dma_start(c_cur[:], c0_in[:])
                hT_cur = bh.tile([128, 16 * B], DT, tag="hT")
                nc.sync.dma_start(
                    hT_cur[:].rearrange("p (kt n) -> p kt n", kt=16),
                    h0T_in[:].rearrange("(kt p) n -> p kt n", p=128),
                )

                for t in range(T):
                    gx_t = bgx.tile([B, GC], DT, tag="gx")
                    nc.sync.dma_start(gx_t[:], gx_d[B * t:B * (t + 1), :])

                    # gate columns: [g(0:256) | i(256:512) | f(512:768) | o(768:1024)]
                    ps = bpg.tile([B, GC], f32)
                    nc.tensor.matmul(ps[:, 0:512], lhsT=(id32[:]), rhs=(gx_t[:, 0:512]),
                                     start=True, stop=False)
                    for kt in range(16):
                        nc.tensor.matmul(ps[:, 0:512], lhsT=hT_cur[:, B * kt:B * (kt + 1)],
                                         rhs=whh_sb[:, GC * kt:GC * kt + 512],
                                         start=False, stop=(kt == 15))
                    nc.tensor.matmul(ps[:, 512:1024], lhsT=(id32[:]), rhs=(gx_t[:, 512:1024]),
                                     start=True, stop=False)
                    for kt in range(16):
                        nc.tensor.matmul(ps[:, 512:1024], lhsT=hT_cur[:, B * kt:B * (kt + 1)],
                                         rhs=whh_sb[:, GC * kt + 512:GC * (kt + 1)],
                                         start=False, stop=(kt == 15))

                    tng = bel.tile([B, U], f32, tag="tng")
                    nc.scalar.activation(tng[:], ps[:, 0:256],
                                         mybir.ActivationFunctionType.Tanh)
                    sgi = bel.tile([B, U], f32, tag="sgi")
                    nc.scalar.activation(sgi[:], ps[:, 256:512],
                                         mybir.ActivationFunctionType.Sigmoid)
                    t1 = bel.tile([B, U], f32, tag="t1")
                    nc.vector.tensor_mul(t1[:], sgi[:], tng[:])
                    sfo = bel.tile([B, 512], f32, tag="sfo")
                    nc.scalar.activation(sfo[:], ps[:, 512:1024],
                                         mybir.ActivationFunctionType.Sigmoid)
                    t2 = bel.tile([B, U], f32, tag="t2")
                    nc.vector.tensor_mul(t2[:], sfo[:, 0:256], c_cur[:])
                    c_new = bcell.tile([B, U], f32, tag="c")
                    nc.vector.tensor_add(c_new[:], t1[:], t2[:])
                    tnc = bel.tile([B, U], f32, tag="tnc")
                    nc.scalar.activation(tnc[:], c_new[:],
                                         mybir.ActivationFunctionType.Tanh)
                    h16 = bel.tile([B, U], DT, tag="h16")
                    nc.vector.tensor_mul(h16[:], sfo[:, 256:512], tnc[:])

                    tp = bpt.tile([128, 64], DT)
                    nc.tensor.transpose((tp[:, 0:32]), (h16[:, 0:128]), (id32[:]))
                    nc.tensor.transpose((tp[:, 32:64]), (h16[:, 128:256]), (id32[:]))
                    hT_sb = bel.tile([128, 64], DT, tag="hTsb")
                    nc.vector.tensor_copy(hT_sb[:], tp[:])

                    hb = dramcc.tile([2 * 128, B], DT, tag="hb")
                    nc.sync.dma_start(
                        hb[:].rearrange("(a p) f -> p a f", p=128),
                        hT_sb[:].rearrange("p (a f) -> p a f", a=2),
                    )
                    hfull = dramcc.tile([N_CORES * 2 * 128, B], DT, tag="hfull")
                    nc.gpsimd.collective_compute(
                        "AllGather", mybir.AluOpType.bypass,
                        ins=[hb.opt()], outs=[hfull.opt()],
                        replica_groups=[list(range(N_CORES))],
                    )
                    hT_new = bh.tile([128, 16 * B], DT, tag="hT")
                    hTn_v = hT_new[:].rearrange("p (kt n) -> p kt n", kt=16)
                    hf_v = hfull[:].rearrange("(kt p) n -> p kt n", p=128)
                    nc.sync.dma_start(hTn_v[:, 0:8], hf_v[:, 0:8])
                    nc.sync.dma_start(hTn_v[:, 8:16], hf_v[:, 8:16])

                    # decoder for step t (h_t just gathered), overlaps next AG
                    dps = bpd.tile([B, S], f32)
                    for kt in range(16):
                        nc.tensor.matmul(dps[:], lhsT=(hT_new[:, B * kt:B * (kt + 1)]),
                                         rhs=(dec_sb[:, S * kt:S * (kt + 1)]),
                                         start=(kt == 0), stop=(kt == 15))
                    outsb = bo.tile([B, S], f32, tag="outsb")
                    nc.vector.tensor_add(outsb[:], dps[:], decb_sb[:])
                    nc.sync.dma_start(out_d[B * t:B * (t + 1), :], outsb[:])

                    if t == T - 1:
                        h32 = bel.tile([B, U], f32, tag="h32")
                        nc.vector.tensor_mul(h32[:], sfo[:, 256:512], tnc[:])
                        nc.sync.dma_start(hout_d[:], h32[:])
                        nc.sync.dma_start(cout_d[:], c_new[:])

                    c_cur = c_new
                    hT_cur = hT_new

    nc.compile()
    return nc


def _prep_inputs(input_1, input_2, h0, c0, routers_info,
                 m1_W, m1_b, m2_W, m2_b, enc_W, enc_b,
                 lstm_Wih, lstm_Whh, lstm_bih, lstm_bhh, dec_W, dec_b):
    T = input_1.shape[1]
    TPC = T // N_CORES
    TOK = TPC * B

    f32 = np.float32
    input_1 = np.asarray(input_1, f32)
    input_2 = np.asarray(input_2, f32)
    routers = np.asarray(routers_info).astype(np.int64)

    # folded expert table T4[s] = [[m1_W^T @ We_u], [m2_W^T @ We_v], [bias]]
    enc_W = np.asarray(enc_W, f32)      # [S, 20, H]
    enc_b = np.asarray(enc_b, f32)      # [S, H]
    We_u = enc_W[:, 0:10, :]            # [S, 10, H]
    We_v = enc_W[:, 10:20, :]
    m1_W = np.asarray(m1_W, f32)        # [10, 2]
    m2_W = np.asarray(m2_W, f32)        # [10, 1]
    T4 = np.empty((S, 4, H), f32)
    T4[:, 0:2, :] = np.einsum("fr,sfh->srh", m1_W, We_u)
    T4[:, 2:3, :] = np.einsum("fr,sfh->srh", m2_W, We_v)
    T4[:, 3, :] = (np.einsum("f,sfh->sh", np.asarray(m1_b, f32), We_u)
                   + np.einsum("f,sfh->sh", np.asarray(m2_b, f32), We_v)
                   + enc_b)
    expt4 = np.ascontiguousarray(T4.reshape(S * 4, H)).astype(NPADT)

    # per-core gate-row order [i|f|o|g] for this core's 256 units
    Wih = np.asarray(lstm_Wih, f32)
    Whh = np.asarray(lstm_Whh, f32)
    bihh = np.asarray(lstm_bih, f32) + np.asarray(lstm_bhh, f32)
    dec_WT = np.ascontiguousarray(np.asarray(dec_W, f32).T).astype(NPDT)  # [H, S]
    decb = np.asarray(dec_b, f32).reshape(1, S)

    h0 = np.asarray(h0, f32)
    c0 = np.asarray(c0, f32)
    h0T = np.ascontiguousarray(h0.T).astype(NPDT)  # [H, B]

    # f4[n] = [x1_0, x1_1, x2_0, 1]
    x1 = input_1  # [B, T, 2]
    x2 = input_2  # [B, T, 1]

    in_maps = []
    for k in range(N_CORES):
        uk = np.arange(U * k, U * (k + 1))
        rows = np.concatenate([2 * H + uk, uk, H + uk, 3 * H + uk])  # g,i,f,o
        wihT = np.ascontiguousarray(Wih[rows].T).astype(NPADT)  # [H, GC]
        whhT = np.ascontiguousarray(Whh[rows].T).astype(NPDT)
        biask = np.ascontiguousarray(bihh[rows].reshape(1, GC))

        ts0 = TPC * k
        # block-diag fused inputs: bd[4j+r, 32g+j] = f4[b=j, t=ts0+g, r]
        bd = np.zeros((128, TOK), f32)
        offs = np.zeros((128, TPC), np.int32)
        for g in range(TPC):
            t = ts0 + g
            for j in range(B):
                bd[4 * j + 0, B * g + j] = x1[j, t, 0]
                bd[4 * j + 1, B * g + j] = x1[j, t, 1]
                bd[4 * j + 2, B * g + j] = x2[j, t, 0]
                bd[4 * j + 3, B * g + j] = 1.0
                s = int(routers[j, t])
                offs[4 * j + 0, g] = 4 * s + 0
                offs[4 * j + 1, g] = 4 * s + 1
                offs[4 * j + 2, g] = 4 * s + 2
                offs[4 * j + 3, g] = 4 * s + 3

        in_maps.append({
            "expt4": expt4,
            "bd": bd.astype(NPADT),
            "offs": offs,
            "wihT": wihT,
            "whhT": whhT,
            "biask": biask,
            "decWT": dec_WT,
            "decb": decb,
            "h0T": h0T,
            "c0k": np.ascontiguousarray(c0[:, uk]),
        })
    return in_maps, T


def kernel(**inputs):
    in_maps, T = _prep_inputs(**inputs)
    if T not in _BUILD_CACHE:
        _BUILD_CACHE[T] = _build(T)
    nc = _BUILD_CACHE[T]

    res = run_bass_kernel_spmd(nc, in_maps, core_ids=list(range(N_CORES)))
    kernel.last_results = res

    out_tb = res.results[0]["out"]            # [T*B, S], rows t-major
    NTOK = T * B
    output = np.ascontiguousarray(
        out_tb.reshape(T, B, S).transpose(1, 0, 2).reshape(NTOK, S))
    hT = np.concatenate([res.results[k]["h_out"] for k in range(N_CORES)], axis=1)
    cT = np.concatenate([res.results[k]["c_out"] for k in range(N_CORES)], axis=1)
    return output.astype(np.float32), hT.astype(np.float32), cT.astype(np.float32)
